# revision 3
# baseline (speedup 1.0000x reference)
"""ConvS5 SSM kernel for Trainium2 (8 NeuronCores, data-parallel over batch).

Math per batch element b (one NeuronCore each):
  Bus = conv3x3(u, step*B_r) + i*conv3x3(u, step*B_i)       (U=128 -> P=256)
  x_l = A_bar * x_{l-1} + Bus_l   (diagonal complex scan, x_{-1} = x0)
  ys  = 2*(conv3x3(x_r, C_r) - conv3x3(x_i, C_i)) + depthwise3x3(u, D)
  out = gelu(groupnorm(ys))                                  (P=256 -> U=128)

Device mapping: channels on SBUF partitions, pixels on the free dim, images
zero-padded to 34x34 so conv taps are plain offset windows.  Each 3x3 conv is
9 shifted matmuls accumulated in PSUM (bf16 operands, fp32 accumulate).  The
scan recurrence A*x is 2 extra diagonal matmuls into the same PSUM group, so
the scan costs no vector-engine time and the PSUM result IS the new state.
GroupNorm stats come from activation accum_out (sum / sum-of-squares), group
reduction/broadcast via tiny fp32 mask matmuls, and the final
normalize+gelu is a single fused scalar-engine activation per image.
"""

import sys

for _p in ("/opt/trn_rl_repo",):
    if _p not in sys.path:
        sys.path.insert(0, _p)

import numpy as np
import ml_dtypes


def _install_ntff_hook_shim():
    """Provide antenv.axon_hooks (absent in this image) so trace=True works."""
    import types
    try:
        import antenv.axon_hooks  # noqa: F401
        return
    except ImportError:
        pass
    mod = types.ModuleType("antenv.axon_hooks")
    state = {"hook": None, "tried": False}

    def set_axon_ntff_profile_hook(h):
        state["hook"] = h

    def get_axon_ntff_profile_hook():
        if state["hook"] is None and not state["tried"]:
            state["tried"] = True
            try:
                from trn_agent_boot.trn_boot import _ntff_profile_via_ctypes
                state["hook"] = _ntff_profile_via_ctypes(
                    "/opt/axon/libaxon_pjrt.so")
            except Exception:
                state["hook"] = None
        return state["hook"]

    mod.set_axon_ntff_profile_hook = set_axon_ntff_profile_hook
    mod.get_axon_ntff_profile_hook = get_axon_ntff_profile_hook
    try:
        import antenv
        antenv.axon_hooks = mod
    except ImportError:
        pass
    sys.modules["antenv.axon_hooks"] = mod


_install_ntff_hook_shim()

import concourse.bass as bass
import concourse.bacc as bacc
import concourse.tile as tile
import concourse.mybir as mybir
from concourse import masks
from concourse.bass_utils import run_bass_kernel_spmd

F32 = mybir.dt.float32
BF16 = mybir.dt.bfloat16
AF = mybir.ActivationFunctionType
OP = mybir.AluOpType

L, BSZ, H, W, U, P = 16, 8, 32, 32, 128, 256
HP, WP = H + 2, W + 2  # padded image
IMG = H * W            # 1024 pixels
PADIMG = HP * WP       # 1156
EPS = 1e-5
GSZ = 4                # channels per group
NG = 32                # groups


def _build(tc, d, Lsteps):
    nc = tc.nc
    uT_v = d["uT"].rearrange("p (l h w) -> p l h w", l=Lsteps, h=HP, w=WP)
    st_v = [[[d["st"][par][ri][half].rearrange("p (h w) -> p h w", h=HP, w=WP)
              for half in (0, 1)] for ri in (0, 1)] for par in (0, 1)]
    ident = d["ident"]

    def wb_t(tap, ri, half):
        o = ((tap * 2 + ri) * 2 + half) * 128
        return d["wb_sb"][:, o:o + 128]

    def wc_t(tap, kb):
        o = (tap * 4 + kb) * 128
        return d["wc_sb"][:, o:o + 128]

    def wd_t(tap):
        return d["wd_sb"][:, tap * 128:(tap + 1) * 128]

    def wa_t(slot):
        return d["wa_sb"][:, slot * 128:(slot + 1) * 128]

    # ---- pools ----
    import contextlib
    ctx = contextlib.ExitStack()
    inp = ctx.enter_context(tc.tile_pool(name="inp", bufs=4))
    ysp = ctx.enter_context(tc.tile_pool(name="ysp", bufs=3))
    hp_ = ctx.enter_context(tc.tile_pool(name="hp", bufs=2))
    sqp = ctx.enter_context(tc.tile_pool(name="sqp", bufs=2))
    smp = ctx.enter_context(tc.tile_pool(name="smp", bufs=4))
    outp = ctx.enter_context(tc.tile_pool(name="outp", bufs=4))
    xlp = ctx.enter_context(tc.tile_pool(name="xlp", bufs=2))
    psA = ctx.enter_context(tc.tile_pool(name="psA", bufs=3, space="PSUM"))
    psC = ctx.enter_context(tc.tile_pool(name="psC", bufs=2, space="PSUM"))
    psT = ctx.enter_context(tc.tile_pool(name="psT", bufs=2, space="PSUM"))
    psG = ctx.enter_context(tc.tile_pool(name="psG", bufs=1, space="PSUM"))

    # ---- constants / zero-init ----
    masks.make_identity(nc, ident)
    nc.gpsimd.memset(d["uT"], 0.0)
    for par in (0, 1):
        for ri in (0, 1):
            for half in (0, 1):
                nc.gpsimd.memset(d["st"][par][ri][half], 0.0)

    # ---- load + transpose x0 into state parity 1 (real part; imag stays 0) ----
    for t in range(8):
        x0in = inp.tile([128, 256], F32, name="x0in")
        nc.sync.dma_start(x0in, d["x0"].ap()[128 * t:128 * (t + 1), :])
        for half in (0, 1):
            pt = psT.tile([128, 128], F32, name="psT")
            nc.tensor.transpose(pt, x0in[:, 128 * half:128 * (half + 1)], ident)
            dst = st_v[1][0][half][:, 1 + 4 * t:5 + 4 * t, 1:33]
            nc.vector.tensor_copy(dst, pt.rearrange("p (a b) -> p a b", a=4))

    # ---- load + transpose u into padded channel-major layout (bf16) ----
    for t in range(Lsteps * 8):
        l, j = t // 8, t % 8
        uin = inp.tile([128, 128], F32, name="uin")
        nc.sync.dma_start(uin, d["u"].ap()[128 * t:128 * (t + 1), :])
        pt = psT.tile([128, 128], F32, name="psT")
        nc.tensor.transpose(pt, uin, ident)
        dst = uT_v[:, l, 1 + 4 * j:5 + 4 * j, 1:33]
        nc.vector.tensor_copy(dst, pt.rearrange("p (a b) -> p a b", a=4))

    # ---- main loop over time steps ----
    for l in range(Lsteps):
        cur, prv = l % 2, 1 - (l % 2)
        # B conv + scan recurrence -> new state (both 512-pixel chunks)
        for c in (0, 1):
            r0 = 16 * c
            for ri in (0, 1):
                for half in (0, 1):
                    ps = psA.tile([128, 512], F32, name="psA")
                    for tap in range(9):
                        dh, dw = tap // 3, tap % 3
                        rhs = uT_v[:, l, dh + r0:dh + r0 + 16, dw:dw + 32]
                        nc.tensor.matmul(ps, wb_t(tap, ri, half), rhs,
                                         start=(tap == 0), stop=False)
                    if ri == 0:
                        s1, s2 = wa_t(0 + half), wa_t(4 + half)  # Ar, -Ai
                    else:
                        s1, s2 = wa_t(2 + half), wa_t(0 + half)  # Ai, Ar
                    rhs_r = st_v[prv][0][half][:, 1 + r0:17 + r0, 1:33]
                    rhs_i = st_v[prv][1][half][:, 1 + r0:17 + r0, 1:33]
                    nc.tensor.matmul(ps, s1, rhs_r, start=False, stop=False)
                    nc.tensor.matmul(ps, s2, rhs_i, start=False, stop=True)
                    dst = st_v[cur][ri][half][:, 1 + r0:17 + r0, 1:33]
                    ps_v = ps.rearrange("p (a b) -> p a b", a=16)
                    if ri == 0:
                        nc.scalar.copy(dst, ps_v)
                    else:
                        nc.vector.tensor_copy(dst, ps_v)
                    if l == Lsteps - 1:
                        nc.vector.tensor_copy(
                            d["xfin"][ri][half][:, 512 * c:512 * (c + 1)], ps)

        # C conv + depthwise D -> ys (needs both state chunks written)
        ys_t = ysp.tile([128, 1024], F32, name="ys_t")
        sq4 = smp.tile([128, 4], F32, name="sq4")
        for c in (0, 1):
            r0 = 16 * c
            pc = psC.tile([128, 512], F32, name="psC")
            first = True
            for tap in range(9):
                dh, dw = tap // 3, tap % 3
                for kb in range(4):
                    ri, half = kb // 2, kb % 2
                    rhs = st_v[cur][ri][half][:, dh + r0:dh + r0 + 16, dw:dw + 32]
                    nc.tensor.matmul(pc, wc_t(tap, kb), rhs,
                                     start=first, stop=False)
                    first = False
                rhs_u = uT_v[:, l, dh + r0:dh + r0 + 16, dw:dw + 32]
                nc.tensor.matmul(pc, wd_t(tap), rhs_u,
                                 start=False, stop=(tap == 8))
            nc.scalar.activation(ys_t[:, 512 * c:512 * (c + 1)], pc, AF.Copy,
                                 accum_out=sq4[:, c:c + 1])
            sqs = sqp.tile([128, 512], F32, name="sqs")
            nc.scalar.activation(sqs, pc, AF.Square,
                                 accum_out=sq4[:, 2 + c:3 + c])

        # GroupNorm stats -> per-partition affine -> fused gelu
        sq2 = smp.tile([128, 2], F32, name="sq2")
        nc.vector.tensor_add(sq2[:, 0:1], sq4[:, 0:1], sq4[:, 1:2])
        nc.vector.tensor_add(sq2[:, 1:2], sq4[:, 2:3], sq4[:, 3:4])
        pg = psG.tile([32, 2], F32, name="psg", tag="gn")
        nc.tensor.matmul(pg, d["gmask_sb"], sq2, start=True, stop=True)
        mex = smp.tile([32, 8], F32, name="mex")
        nc.scalar.activation(mex[:, 0:2], pg, AF.Copy, scale=1.0 / 4096.0)
        nc.vector.tensor_mul(mex[:, 2:3], mex[:, 0:1], mex[:, 0:1])
        nc.vector.tensor_sub(mex[:, 3:4], mex[:, 1:2], mex[:, 2:3])
        nc.scalar.activation(mex[:, 4:5], mex[:, 3:4], AF.Sqrt, bias=EPS)
        nc.vector.reciprocal(mex[:, 5:6], mex[:, 4:5])
        nc.vector.scalar_tensor_tensor(mex[:, 6:7], mex[:, 0:1], -1.0,
                                       mex[:, 5:6], op0=OP.mult, op1=OP.mult)
        ab = smp.tile([32, 2], F32, name="ab")
        nc.vector.tensor_copy(ab[:, 0:1], mex[:, 5:6])
        nc.vector.tensor_copy(ab[:, 1:2], mex[:, 6:7])
        pb = psG.tile([128, 2], F32, name="psb", tag="gn")
        nc.tensor.matmul(pb, d["gmaskt_sb"], ab, start=True, stop=True)
        abf = smp.tile([128, 2], F32, name="abf")
        nc.vector.tensor_scalar(abf[:, 0:1], pb[:, 0:1],
                                d["gnsb_sb"][:, 0:1], None, op0=OP.mult)
        nc.vector.tensor_scalar(abf[:, 1:2], pb[:, 1:2],
                                d["gnsb_sb"][:, 0:1], d["gnsb_sb"][:, 1:2],
                                op0=OP.mult, op1=OP.add)
        h_t = hp_.tile([128, 1024], F32, name="h_t")
        nc.scalar.activation(h_t, ys_t, AF.Gelu_apprx_tanh,
                             bias=abf[:, 1:2], scale=abf[:, 0:1])

        # transpose back to pixel-major and store
        for j in range(8):
            pt = psT.tile([128, 128], F32, name="psT")
            nc.tensor.transpose(pt, h_t[:, 128 * j:128 * (j + 1)], ident)
            hT = outp.tile([128, 128], F32, name="hT")
            nc.scalar.copy(hT, pt)
            row = l * 1024 + 128 * j
            nc.sync.dma_start(d["ys"].ap()[row:row + 128, :], hT)

    # ---- final state -> (pix, p, ri) interleaved output ----
    for t in range(8):
        xt = xlp.tile([128, 512], F32, name="xt")
        xt_v = xt.rearrange("a (p r) -> a p r", r=2)
        for half in (0, 1):
            for ri in (0, 1):
                pt = psT.tile([128, 128], F32, name="psT")
                nc.tensor.transpose(
                    pt, d["xfin"][ri][half][:, 128 * t:128 * (t + 1)], ident)
                nc.vector.tensor_copy(xt_v[:, 128 * half:128 * (half + 1), ri], pt)
        nc.sync.dma_start(d["xl"].ap()[128 * t:128 * (t + 1), :], xt)
    ctx.close()


def build_nc(Lsteps=L, num_devices=8):
    nc = bacc.Bacc("TRN2", target_bir_lowering=False, debug=False,
                   num_devices=num_devices)
    # register EPS so activation(..., bias=EPS) can lower it to a const AP
    _ct = nc.alloc_sbuf_tensor(f"const-f32-eps", [128, 1], F32)
    nc.gpsimd.memset(_ct.ap(), EPS)
    nc.const_aps.aps[(F32, EPS)] = _ct.ap()
    nc.all_engine_barrier()
    d = {}
    d["u"] = nc.dram_tensor("u", [Lsteps * IMG, U], F32, kind="ExternalInput")
    d["x0"] = nc.dram_tensor("x0", [IMG, P], F32, kind="ExternalInput")
    d["wb"] = nc.dram_tensor("wb", [128, 9 * 2 * 2 * 128], BF16, kind="ExternalInput")
    d["wc"] = nc.dram_tensor("wc", [128, 9 * 4 * 128], BF16, kind="ExternalInput")
    d["wd"] = nc.dram_tensor("wd", [128, 9 * 128], BF16, kind="ExternalInput")
    d["wa"] = nc.dram_tensor("wa", [128, 6 * 128], BF16, kind="ExternalInput")
    d["gmask"] = nc.dram_tensor("gmask", [128, 32], F32, kind="ExternalInput")
    d["gmaskt"] = nc.dram_tensor("gmaskt", [32, 128], F32, kind="ExternalInput")
    d["gnsb"] = nc.dram_tensor("gnsb", [128, 2], F32, kind="ExternalInput")
    d["ys"] = nc.dram_tensor("ys", [Lsteps * IMG, U], F32, kind="ExternalOutput")
    d["xl"] = nc.dram_tensor("xl", [IMG, 2 * P], F32, kind="ExternalOutput")

    with tile.TileContext(nc) as tc:
        cpool_ctx = tc.tile_pool(name="const", bufs=1)
        cpool = cpool_ctx.__enter__()
        d["wb_sb"] = cpool.tile([128, 9 * 2 * 2 * 128], BF16, name="wb_sb")
        d["wc_sb"] = cpool.tile([128, 9 * 4 * 128], BF16, name="wc_sb")
        d["wd_sb"] = cpool.tile([128, 9 * 128], BF16, name="wd_sb")
        d["wa_sb"] = cpool.tile([128, 6 * 128], BF16, name="wa_sb")
        d["gmask_sb"] = cpool.tile([128, 32], F32, name="gmask_sb")
        d["gmaskt_sb"] = cpool.tile([32, 128], F32, name="gmaskt_sb")
        d["gnsb_sb"] = cpool.tile([128, 2], F32, name="gnsb_sb")
        d["ident"] = cpool.tile([128, 128], F32, name="ident")
        d["uT"] = cpool.tile([128, Lsteps * PADIMG], BF16, name="uT")
        d["st"] = [[[cpool.tile([128, PADIMG], BF16,
                                name=f"st{par}{ri}{half}")
                     for half in (0, 1)] for ri in (0, 1)] for par in (0, 1)]
        d["xfin"] = [[cpool.tile([128, IMG], F32, name=f"xfin{ri}{half}")
                      for half in (0, 1)] for ri in (0, 1)]
        for nm in ("wb_sb", "wc_sb", "wd_sb", "wa_sb", "gmask_sb",
                   "gmaskt_sb", "gnsb_sb"):
            nc.sync.dma_start(d[nm], d[nm[:-3]].ap())
        _build(tc, d, Lsteps)
        cpool_ctx.__exit__(None, None, None)
    nc.compile()
    return nc


def prep_weights(Lambda_re, Lambda_im, B_ri, C_ri, log_step, D_kernel,
                 gn_scale, gn_bias):
    f32, bf16 = np.float32, ml_dtypes.bfloat16
    step = np.exp(np.asarray(log_step, f32)).astype(f32)
    lam_re = np.minimum(np.asarray(Lambda_re, f32), -1e-4)
    A_r = (lam_re * step).astype(f32)
    A_i = (np.asarray(Lambda_im, f32) * step).astype(f32)

    B = np.asarray(B_ri, f32) * step[:, None, None, None, None]  # (P,U,3,3,2)
    wb = np.transpose(B, (1, 2, 3, 4, 0))          # (U,3,3,2,P)
    wb = wb.reshape(128, 9, 2, 2, 128).reshape(128, -1).astype(bf16)

    C = np.asarray(C_ri, f32)                       # (U,P,3,3,2)
    Ck = np.transpose(C, (1, 2, 3, 4, 0)).reshape(256, 9, 2, 128)  # (p,tap,ri,u)
    wc = np.empty((128, 9, 4, 128), f32)
    wc[:, :, 0, :] = 2.0 * Ck[0:128, :, 0, :]
    wc[:, :, 1, :] = 2.0 * Ck[128:256, :, 0, :]
    wc[:, :, 2, :] = -2.0 * Ck[0:128, :, 1, :]
    wc[:, :, 3, :] = -2.0 * Ck[128:256, :, 1, :]
    wc = wc.reshape(128, -1).astype(bf16)

    Dk = np.asarray(D_kernel, f32).reshape(9, 128)
    wd = np.zeros((128, 9, 128), f32)
    for t in range(9):
        np.fill_diagonal(wd[:, t, :], Dk[t])
    wd = wd.reshape(128, -1).astype(bf16)

    vals = [A_r[:128], A_r[128:], A_i[:128], A_i[128:], -A_i[:128], -A_i[128:]]
    wa = np.zeros((128, 6, 128), f32)
    for s, v in enumerate(vals):
        np.fill_diagonal(wa[:, s, :], v)
    wa = wa.reshape(128, -1).astype(bf16)

    gmask = (np.arange(128)[:, None] // GSZ == np.arange(NG)[None, :]).astype(f32)
    gmaskt = np.ascontiguousarray(gmask.T)
    gnsb = np.stack([np.asarray(gn_scale, f32), np.asarray(gn_bias, f32)], 1)
    return dict(wb=wb, wc=wc, wd=wd, wa=wa, gmask=gmask, gmaskt=gmaskt,
                gnsb=np.ascontiguousarray(gnsb))


def make_in_maps(input_sequence, x0, weights):
    f32 = np.float32
    inp = np.asarray(input_sequence, f32)
    x0n = np.asarray(x0, f32)
    in_maps = []
    for c in range(BSZ):
        m = dict(weights)
        m["u"] = np.ascontiguousarray(inp[:, c].reshape(L * IMG, U))
        m["x0"] = np.ascontiguousarray(x0n[c].reshape(IMG, P))
        in_maps.append(m)
    return in_maps


_CACHE = {}


def get_nc():
    if "nc" not in _CACHE:
        _CACHE["nc"] = build_nc(L, 8)
    return _CACHE["nc"]


def run(inputs, trace=False, **kw):
    nc = get_nc()
    weights = prep_weights(
        inputs["Lambda_re"], inputs["Lambda_im"], inputs["B_ri"],
        inputs["C_ri"], inputs["log_step"], inputs["D_kernel"],
        inputs["gn_scale"], inputs["gn_bias"])
    in_maps = make_in_maps(inputs["input_sequence"], inputs["x0"], weights)
    res = run_bass_kernel_spmd(nc, in_maps, list(range(8)), trace=trace, **kw)
    ys_full = np.stack(
        [r["ys"].reshape(L, H, W, U) for r in res.results], axis=1)
    xl_full = np.stack(
        [r["xl"].reshape(H, W, P, 2) for r in res.results], axis=0)
    return (xl_full, ys_full), res


def kernel(**inputs):
    (xl_full, ys_full), _ = run(inputs, trace=False)
    return xl_full, ys_full


# revision 19
# speedup vs baseline: 1.0657x; 1.0657x over previous
"""ConvS5 SSM kernel for Trainium2 (8 NeuronCores, data-parallel over batch).

Math per batch element b (one NeuronCore each):
  Bus = conv3x3(u, step*B_r) + i*conv3x3(u, step*B_i)       (U=128 -> P=256)
  x_l = A_bar * x_{l-1} + Bus_l   (diagonal complex scan, x_{-1} = x0)
  ys  = 2*(conv3x3(x_r, C_r) - conv3x3(x_i, C_i)) + depthwise3x3(u, D)
  out = gelu(groupnorm(ys))                                  (P=256 -> U=128)

Device mapping: channels on SBUF partitions, pixels on the free dim, images
zero-padded to 34x34 so conv taps are plain offset windows.  Each 3x3 conv is
9 shifted matmuls accumulated in PSUM (bf16 operands, fp32 accumulate).  The
scan recurrence A*x is 2 extra diagonal matmuls into the same PSUM group, so
the scan costs no vector-engine time and the PSUM result IS the new state.
GroupNorm stats come from activation accum_out (sum / sum-of-squares), group
reduction/broadcast via tiny fp32 mask matmuls, and the final
normalize+gelu is a single fused scalar-engine activation per image.
"""

import sys

for _p in ("/opt/trn_rl_repo",):
    if _p not in sys.path:
        sys.path.insert(0, _p)

import numpy as np
import ml_dtypes


def _install_ntff_hook_shim():
    """Provide antenv.axon_hooks (absent in this image) so trace=True works."""
    import types
    try:
        import antenv.axon_hooks  # noqa: F401
        return
    except ImportError:
        pass
    mod = types.ModuleType("antenv.axon_hooks")
    state = {"hook": None, "tried": False}

    def set_axon_ntff_profile_hook(h):
        state["hook"] = h

    def get_axon_ntff_profile_hook():
        if state["hook"] is None and not state["tried"]:
            state["tried"] = True
            try:
                from trn_agent_boot.trn_boot import _ntff_profile_via_ctypes
                state["hook"] = _ntff_profile_via_ctypes(
                    "/opt/axon/libaxon_pjrt.so")
            except Exception:
                state["hook"] = None
        return state["hook"]

    mod.set_axon_ntff_profile_hook = set_axon_ntff_profile_hook
    mod.get_axon_ntff_profile_hook = get_axon_ntff_profile_hook
    try:
        import antenv
        antenv.axon_hooks = mod
    except ImportError:
        pass
    sys.modules["antenv.axon_hooks"] = mod


_install_ntff_hook_shim()

import concourse.bass as bass
import concourse.bacc as bacc
import concourse.tile as tile
import concourse.mybir as mybir
from concourse import masks
from concourse.bass_utils import run_bass_kernel_spmd

F32 = mybir.dt.float32
BF16 = mybir.dt.bfloat16
AF = mybir.ActivationFunctionType
OP = mybir.AluOpType

L, BSZ, H, W, U, P = 16, 8, 32, 32, 128, 256
HP, WP = H + 2, W + 2  # padded image
IMG = H * W            # 1024 pixels
PADIMG = HP * WP       # 1156
EPS = 1e-5
GSZ = 4                # channels per group
NG = 32                # groups


def _build(tc, d, Lsteps):
    nc = tc.nc
    uT_v = d["uT"].rearrange("p (l h w) -> p l h w", l=Lsteps, h=HP, w=WP)
    st_v = [[[d["st"][par][ri][half].rearrange("p (h w) -> p h w", h=HP, w=WP)
              for half in (0, 1)] for ri in (0, 1)] for par in (0, 1)]
    ident = d["ident"]

    def wb_t(tap, ri, half):
        o = ((tap * 2 + ri) * 2 + half) * 128
        return d["wb_sb"][:, o:o + 128]

    def wc_t(tap, kb):
        o = (tap * 4 + kb) * 128
        return d["wc_sb"][:, o:o + 128]

    def wa_t(slot):
        return d["wa_sb"][:, slot * 128:(slot + 1) * 128]

    # ---- pools ----
    import contextlib
    ctx = contextlib.ExitStack()
    inp = ctx.enter_context(tc.tile_pool(name="inp", bufs=4))
    ysp = ctx.enter_context(tc.tile_pool(name="ysp", bufs=3))
    hp_ = ctx.enter_context(tc.tile_pool(name="hp", bufs=2))
    dup = ctx.enter_context(tc.tile_pool(name="dup", bufs=3))
    sqp = ctx.enter_context(tc.tile_pool(name="sqp", bufs=2))
    smp = ctx.enter_context(tc.tile_pool(name="smp", bufs=4))
    outp = ctx.enter_context(tc.tile_pool(name="outp", bufs=4))
    xlp = ctx.enter_context(tc.tile_pool(name="xlp", bufs=2))
    psA = ctx.enter_context(tc.tile_pool(name="psA", bufs=3, space="PSUM"))
    psC = ctx.enter_context(tc.tile_pool(name="psC", bufs=2, space="PSUM"))
    psT = ctx.enter_context(tc.tile_pool(name="psT", bufs=2, space="PSUM"))
    psG = ctx.enter_context(tc.tile_pool(name="psG", bufs=1, space="PSUM"))

    # ---- constants / zero-init ----
    masks.make_identity(nc, ident)
    for l in range(Lsteps):
        nc.vector.memset(d["uT"][:, l * PADIMG:(l + 1) * PADIMG], 0.0)
    for par in (0, 1):
        for ri in (0, 1):
            for half in (0, 1):
                nc.vector.memset(d["st"][par][ri][half], 0.0)

    # ---- load x0, cast, xbar-transpose into state parity 1 (imag stays 0) ----
    for t in range(8):
        x0in = inp.tile([128, 256], F32, name="x0in")
        nc.sync.dma_start(x0in, d["x0"].ap()[128 * t:128 * (t + 1), :])
        for half in (0, 1):
            pt = psT.tile([128, 128], F32, name="psT")
            nc.tensor.transpose(pt, x0in[:, 128 * half:128 * (half + 1)], ident)
            dst = st_v[1][0][half][:, 1 + 4 * t:5 + 4 * t, 1:33]
            nc.vector.tensor_copy(dst, pt.rearrange("p (a b) -> p a b", a=4))

    # ---- load u, cast, xbar-transpose into padded channel-major (bf16) ----
    for t in range(Lsteps * 8):
        l, j = t // 8, t % 8
        uin = inp.tile([128, 128], F32, name="uin")
        nc.sync.dma_start(uin, d["u"].ap()[128 * t:128 * (t + 1), :])
        pt = psT.tile([128, 128], F32, name="psT")
        nc.tensor.transpose(pt, uin, ident)
        dst = uT_v[:, l, 1 + 4 * j:5 + 4 * j, 1:33]
        nc.vector.tensor_copy(dst, pt.rearrange("p (a b) -> p a b", a=4))

    # ---- main loop over time steps ----
    for l in range(Lsteps):
        cur, prv = l % 2, 1 - (l % 2)
        # B conv + scan recurrence -> new state (both 512-pixel chunks)
        for c in (0, 1):
            r0 = 16 * c
            for ri in (0, 1):
                for half in (0, 1):
                    ps = psA.tile([128, 512], F32, name="psA")
                    for tap in range(9):
                        dh, dw = tap // 3, tap % 3
                        rhs = uT_v[:, l, dh + r0:dh + r0 + 16, dw:dw + 32]
                        nc.tensor.matmul(ps, wb_t(tap, ri, half), rhs,
                                         start=(tap == 0), stop=False)
                    if ri == 0:
                        s1, s2 = wa_t(0 + half), wa_t(4 + half)  # Ar, -Ai
                    else:
                        s1, s2 = wa_t(2 + half), wa_t(0 + half)  # Ai, Ar
                    rhs_r = st_v[prv][0][half][:, 1 + r0:17 + r0, 1:33]
                    rhs_i = st_v[prv][1][half][:, 1 + r0:17 + r0, 1:33]
                    nc.tensor.matmul(ps, s1, rhs_r, start=False, stop=False)
                    nc.tensor.matmul(ps, s2, rhs_i, start=False, stop=True)
                    dst = st_v[cur][ri][half][:, 1 + r0:17 + r0, 1:33]
                    ps_v = ps.rearrange("p (a b) -> p a b", a=16)
                    # scan-critical: keep off DVE (DVE carries bulk Du work)
                    nc.scalar.copy(dst, ps_v)
                    if l == Lsteps - 1:
                        nc.vector.tensor_copy(
                            d["xfin"][ri][half][:, 512 * c:512 * (c + 1)], ps)

        # C conv on PE; depthwise D on DVE; fuse add + stats into DVE ops
        ys_t = ysp.tile([128, 1024], F32, name="ys_t")
        sq4 = smp.tile([128, 4], F32, name="sq4")
        for c in (0, 1):
            r0 = 16 * c
            pc = psC.tile([128, 512], F32, name="psC")
            first = True
            for tap in range(9):
                dh, dw = tap // 3, tap % 3
                for kb in range(4):
                    ri, half = kb // 2, kb % 2
                    rhs = st_v[cur][ri][half][:, dh + r0:dh + r0 + 16, dw:dw + 32]
                    nc.tensor.matmul(pc, wc_t(tap, kb), rhs,
                                     start=first, stop=(tap == 8 and kb == 3))
                    first = False
            du = dup.tile([128, 512], F32, name="du_t")
            du_v = du.rearrange("p (a b) -> p a b", a=16)
            for tap in range(9):
                dh, dw = tap // 3, tap % 3
                u_w = uT_v[:, l, dh + r0:dh + r0 + 16, dw:dw + 32]
                dsc = d["dsc_sb"][:, tap:tap + 1]
                if tap == 0:
                    nc.vector.tensor_scalar(du_v, u_w, dsc, None, op0=OP.mult)
                else:
                    nc.vector.scalar_tensor_tensor(du_v, u_w, dsc, du_v,
                                                   op0=OP.mult, op1=OP.add)
            ys_chunk = ys_t[:, 512 * c:512 * (c + 1)]
            nc.vector.scalar_tensor_tensor(ys_chunk, pc, 1.0, du,
                                           op0=OP.mult, op1=OP.add,
                                           accum_out=sq4[:, c:c + 1])
            sqs = sqp.tile([128, 512], F32, name="sqs")
            nc.vector.scalar_tensor_tensor(sqs, ys_chunk, 1.0, ys_chunk,
                                           op0=OP.mult, op1=OP.mult,
                                           accum_out=sq4[:, 2 + c:3 + c])

        # GroupNorm stats -> per-partition affine -> fused gelu
        sq2 = smp.tile([128, 2], F32, name="sq2")
        nc.vector.tensor_add(sq2[:, 0:1], sq4[:, 0:1], sq4[:, 1:2])
        nc.vector.tensor_add(sq2[:, 1:2], sq4[:, 2:3], sq4[:, 3:4])
        pg = psG.tile([32, 2], F32, name="psg", tag="gn")
        nc.tensor.matmul(pg, d["gmask_sb"], sq2, start=True, stop=True)
        mex = smp.tile([32, 8], F32, name="mex")
        nc.scalar.activation(mex[:, 0:2], pg, AF.Copy, scale=1.0 / 4096.0)
        nc.vector.tensor_mul(mex[:, 2:3], mex[:, 0:1], mex[:, 0:1])
        nc.vector.tensor_sub(mex[:, 3:4], mex[:, 1:2], mex[:, 2:3])
        nc.scalar.activation(mex[:, 4:5], mex[:, 3:4], AF.Sqrt, bias=EPS)
        nc.vector.reciprocal(mex[:, 5:6], mex[:, 4:5])
        nc.vector.scalar_tensor_tensor(mex[:, 6:7], mex[:, 0:1], -1.0,
                                       mex[:, 5:6], op0=OP.mult, op1=OP.mult)
        ab = smp.tile([32, 2], F32, name="ab")
        nc.vector.tensor_copy(ab[:, 0:1], mex[:, 5:6])
        nc.vector.tensor_copy(ab[:, 1:2], mex[:, 6:7])
        pb = psG.tile([128, 2], F32, name="psb", tag="gn")
        nc.tensor.matmul(pb, d["gmaskt_sb"], ab, start=True, stop=True)
        abf = smp.tile([128, 2], F32, name="abf")
        nc.vector.tensor_scalar(abf[:, 0:1], pb[:, 0:1],
                                d["gnsb_sb"][:, 0:1], None, op0=OP.mult)
        nc.vector.tensor_scalar(abf[:, 1:2], pb[:, 1:2],
                                d["gnsb_sb"][:, 0:1], d["gnsb_sb"][:, 1:2],
                                op0=OP.mult, op1=OP.add)
        h_t = hp_.tile([128, 1024], F32, name="h_t")
        nc.scalar.activation(h_t, ys_t, AF.Gelu_apprx_tanh,
                             bias=abf[:, 1:2], scale=abf[:, 0:1])

        # transpose back to pixel-major and store
        for j in range(8):
            pt = psT.tile([128, 128], F32, name="psT")
            nc.tensor.transpose(pt, h_t[:, 128 * j:128 * (j + 1)], ident)
            hT = outp.tile([128, 128], F32, name="hT")
            nc.scalar.copy(hT, pt)
            row = l * 1024 + 128 * j
            nc.sync.dma_start(d["ys"].ap()[row:row + 128, :], hT)

    # ---- final state -> (pix, p, ri) interleaved output ----
    for t in range(8):
        xt = xlp.tile([128, 512], F32, name="xt")
        xt_v = xt.rearrange("a (p r) -> a p r", r=2)
        for half in (0, 1):
            for ri in (0, 1):
                pt = psT.tile([128, 128], F32, name="psT")
                nc.tensor.transpose(
                    pt, d["xfin"][ri][half][:, 128 * t:128 * (t + 1)], ident)
                nc.vector.tensor_copy(xt_v[:, 128 * half:128 * (half + 1), ri], pt)
        nc.sync.dma_start(d["xl"].ap()[128 * t:128 * (t + 1), :], xt)
    ctx.close()


def build_nc(Lsteps=L, num_devices=8):
    nc = bacc.Bacc("TRN2", target_bir_lowering=False, debug=False,
                   num_devices=num_devices)
    # register EPS so activation(..., bias=EPS) can lower it to a const AP
    _ct = nc.alloc_sbuf_tensor(f"const-f32-eps", [128, 1], F32)
    nc.gpsimd.memset(_ct.ap(), EPS)
    nc.const_aps.aps[(F32, EPS)] = _ct.ap()
    nc.all_engine_barrier()
    d = {}
    d["u"] = nc.dram_tensor("u", [Lsteps * IMG, U], F32, kind="ExternalInput")
    d["x0"] = nc.dram_tensor("x0", [IMG, P], F32, kind="ExternalInput")
    d["wb"] = nc.dram_tensor("wb", [128, 9 * 2 * 2 * 128], BF16, kind="ExternalInput")
    d["wc"] = nc.dram_tensor("wc", [128, 9 * 4 * 128], BF16, kind="ExternalInput")
    d["dsc"] = nc.dram_tensor("dsc", [128, 9], F32, kind="ExternalInput")
    d["wa"] = nc.dram_tensor("wa", [128, 6 * 128], BF16, kind="ExternalInput")
    d["gmask"] = nc.dram_tensor("gmask", [128, 32], F32, kind="ExternalInput")
    d["gmaskt"] = nc.dram_tensor("gmaskt", [32, 128], F32, kind="ExternalInput")
    d["gnsb"] = nc.dram_tensor("gnsb", [128, 2], F32, kind="ExternalInput")
    d["ys"] = nc.dram_tensor("ys", [Lsteps * IMG, U], F32, kind="ExternalOutput")
    d["xl"] = nc.dram_tensor("xl", [IMG, 2 * P], F32, kind="ExternalOutput")

    with tile.TileContext(nc) as tc:
        cpool_ctx = tc.tile_pool(name="const", bufs=1)
        cpool = cpool_ctx.__enter__()
        d["wb_sb"] = cpool.tile([128, 9 * 2 * 2 * 128], BF16, name="wb_sb")
        d["wc_sb"] = cpool.tile([128, 9 * 4 * 128], BF16, name="wc_sb")
        d["dsc_sb"] = cpool.tile([128, 9], F32, name="dsc_sb")
        d["wa_sb"] = cpool.tile([128, 6 * 128], BF16, name="wa_sb")
        d["gmask_sb"] = cpool.tile([128, 32], F32, name="gmask_sb")
        d["gmaskt_sb"] = cpool.tile([32, 128], F32, name="gmaskt_sb")
        d["gnsb_sb"] = cpool.tile([128, 2], F32, name="gnsb_sb")
        d["ident"] = cpool.tile([128, 128], F32, name="ident")
        d["uT"] = cpool.tile([128, Lsteps * PADIMG], BF16, name="uT")
        d["st"] = [[[cpool.tile([128, PADIMG], BF16,
                                name=f"st{par}{ri}{half}")
                     for half in (0, 1)] for ri in (0, 1)] for par in (0, 1)]
        d["xfin"] = [[cpool.tile([128, IMG], F32, name=f"xfin{ri}{half}")
                      for half in (0, 1)] for ri in (0, 1)]
        for nm in ("wb_sb", "wc_sb", "dsc_sb", "wa_sb", "gmask_sb",
                   "gmaskt_sb", "gnsb_sb"):
            nc.sync.dma_start(d[nm], d[nm[:-3]].ap())
        _build(tc, d, Lsteps)
        cpool_ctx.__exit__(None, None, None)
    nc.compile()
    return nc


def prep_weights(Lambda_re, Lambda_im, B_ri, C_ri, log_step, D_kernel,
                 gn_scale, gn_bias):
    f32, bf16 = np.float32, ml_dtypes.bfloat16
    step = np.exp(np.asarray(log_step, f32)).astype(f32)
    lam_re = np.minimum(np.asarray(Lambda_re, f32), -1e-4)
    A_r = (lam_re * step).astype(f32)
    A_i = (np.asarray(Lambda_im, f32) * step).astype(f32)

    B = np.asarray(B_ri, f32) * step[:, None, None, None, None]  # (P,U,3,3,2)
    wb = np.transpose(B, (1, 2, 3, 4, 0))          # (U,3,3,2,P)
    wb = wb.reshape(128, 9, 2, 2, 128).reshape(128, -1).astype(bf16)

    C = np.asarray(C_ri, f32)                       # (U,P,3,3,2)
    Ck = np.transpose(C, (1, 2, 3, 4, 0)).reshape(256, 9, 2, 128)  # (p,tap,ri,u)
    wc = np.empty((128, 9, 4, 128), f32)
    wc[:, :, 0, :] = 2.0 * Ck[0:128, :, 0, :]
    wc[:, :, 1, :] = 2.0 * Ck[128:256, :, 0, :]
    wc[:, :, 2, :] = -2.0 * Ck[0:128, :, 1, :]
    wc[:, :, 3, :] = -2.0 * Ck[128:256, :, 1, :]
    wc = wc.reshape(128, -1).astype(bf16)

    Dk = np.asarray(D_kernel, f32).reshape(9, 128)
    dsc = np.ascontiguousarray(Dk.T)  # (128, 9)

    vals = [A_r[:128], A_r[128:], A_i[:128], A_i[128:], -A_i[:128], -A_i[128:]]
    wa = np.zeros((128, 6, 128), f32)
    for s, v in enumerate(vals):
        np.fill_diagonal(wa[:, s, :], v)
    wa = wa.reshape(128, -1).astype(bf16)

    gmask = (np.arange(128)[:, None] // GSZ == np.arange(NG)[None, :]).astype(f32)
    gmaskt = np.ascontiguousarray(gmask.T)
    gnsb = np.stack([np.asarray(gn_scale, f32), np.asarray(gn_bias, f32)], 1)
    return dict(wb=wb, wc=wc, dsc=dsc, wa=wa, gmask=gmask, gmaskt=gmaskt,
                gnsb=np.ascontiguousarray(gnsb))


def make_in_maps(input_sequence, x0, weights):
    f32 = np.float32
    inp = np.asarray(input_sequence, f32)
    x0n = np.asarray(x0, f32)
    in_maps = []
    for c in range(BSZ):
        m = dict(weights)
        m["u"] = np.ascontiguousarray(inp[:, c].reshape(L * IMG, U))
        m["x0"] = np.ascontiguousarray(x0n[c].reshape(IMG, P))
        in_maps.append(m)
    return in_maps


_CACHE = {}


def get_nc():
    if "nc" not in _CACHE:
        _CACHE["nc"] = build_nc(L, 8)
    return _CACHE["nc"]


def run(inputs, trace=False, **kw):
    nc = get_nc()
    weights = prep_weights(
        inputs["Lambda_re"], inputs["Lambda_im"], inputs["B_ri"],
        inputs["C_ri"], inputs["log_step"], inputs["D_kernel"],
        inputs["gn_scale"], inputs["gn_bias"])
    in_maps = make_in_maps(inputs["input_sequence"], inputs["x0"], weights)
    res = run_bass_kernel_spmd(nc, in_maps, list(range(8)), trace=trace, **kw)
    ys_full = np.stack(
        [r["ys"].reshape(L, H, W, U) for r in res.results], axis=1)
    xl_full = np.stack(
        [r["xl"].reshape(H, W, P, 2) for r in res.results], axis=0)
    return (xl_full, ys_full), res


def kernel(**inputs):
    (xl_full, ys_full), _ = run(inputs, trace=False)
    return xl_full, ys_full
